# revision 25
# baseline (speedup 1.0000x reference)
"""DimeNet edge-update kernel for 8 Trainium2 NeuronCores (v3).

Strategy (graph/data parallel, per the sharding hint):
  - Edges are split into 8 contiguous ranges of 25000 (one per core).
  - Angle triplets are routed (on host) to the core owning their TARGET edge,
    sorted by target, grouped into blocks of EB=16 consecutive target edges.
    With EB=16 a block holds ~80 angles on average (max ~115), so a single
    128-slot sub-block covers a block with no multi-sub accumulation.
  - Host routing also pre-gathers source messages per slot (msgg), builds the
    one-hot scatter S (slot -> target-within-block), and evaluates the tiny
    42->8 angle projection a = ang @ W_angle (0.25% of model FLOPs); S and a
    are packed together (sa_pack).  All heavy FLOPs stay on device.
  - Blocks are processed in GROUPS of 8 (one wide op each for Sa / Gh / Ghd)
    and OCTS of 32 (= 512 edges, one tail tile):
        Sa[j,bk,b,t] = a[j,bk,b] * S[j,bk,t]          (DVE/Pool, 1 op/group)
        G[k,bk,(b,t)] = sum_j msgg[j,k] Sa[j,bk,b,t]  (PE, 1 mm/block)
        Gh[h,...]    = Wsrc^T-contraction of G        (PE, 2 mm/group)
        Ghd          = Gh * dT (d = dist @ Wdist)     (DVE, 1 op/group)
        p1           = Wtgt@x0 + sum_b WbilT_b@Ghd_b  (PE, fused into tail)
    which equals agg + message @ W_tgt with
    agg = segment_sum(einsum('ab,ah,ibh->ai', a, sm, W_bil), tgt),
    sm = (msg[src] @ W_src + b_src) * d[tgt].
  - The edge-wise tail MLP runs fused in bf16 at N=512 tiles,
    software-pipelined in 8 stages across the two following octs so the
    serial PE->Act->Pool dependency chain overlaps with group work.
"""

import sys

sys.path.insert(0, "/opt/trn_rl_repo")

import math
from contextlib import ExitStack

import numpy as np
import ml_dtypes

import concourse.bass as bass
import concourse.tile as tile
from concourse import bacc, mybir

f32 = mybir.dt.float32
f32r = mybir.dt.float32r
bf16 = mybir.dt.bfloat16
i32 = mybir.dt.int32
bf = ml_dtypes.bfloat16

E = 200000
A = 1000000
H = 128
BD = 8
NR = 6
NS = 7
MIN = 128
NCORES = 8
EC = E // NCORES          # 25000 edges per core
EB = 16                   # edges per block
GB = 8                    # blocks per group
OB = 32                   # blocks per oct (= tail tile of 512 edges)
NB = 1568                 # blocks per core (25088 edges padded)
ECP = NB * EB             # 25088
NG = NB // GB             # 196 groups
NO = NB // OB             # 49 octs
P = 128
TB = 512


# ---------------------------------------------------------------- device build

def build_nc(NSUB, has_bsrc, repeat=1, num_devices=NCORES, pool_mod=0):
    """pool_mod: every pool_mod-th group's Sa product runs on gpsimd (Pool);
    0 disables Pool offload."""
    GL = GB * NSUB            # sub-slots per group
    SLOTG = GL * P            # angle slots per group
    nc = bacc.Bacc("TRN2", target_bir_lowering=False, debug=False,
                   enable_asserts=False, num_devices=num_devices)

    dt_ = nc.dram_tensor
    msgg_d = dt_("msgg", [NG * SLOTG, MIN], bf16, kind="ExternalInput").ap()
    sap_d = dt_("sap", [NG * SLOTG, EB + BD], bf16, kind="ExternalInput").ap()
    sohx_d = dt_("sohx", [NG * SLOTG, EB * BD], bf16,
                 kind="ExternalInput").ap()
    distT_d = dt_("distT", [NR, ECP], bf16, kind="ExternalInput").ap()
    msglocT_d = dt_("msglocT", [MIN, ECP], bf16, kind="ExternalInput").ap()
    Wdist_d = dt_("Wdist", [NR, H], bf16, kind="ExternalInput").ap()
    Wsrc_d = dt_("Wsrc", [MIN, H], bf16, kind="ExternalInput").ap()
    WbilT_d = dt_("WbilT", [H, BD * H], bf16, kind="ExternalInput").ap()
    bsrc_d = dt_("bsrc", [1, H], bf16, kind="ExternalInput").ap()
    Wtgt_d = dt_("Wtgt", [MIN, H], bf16, kind="ExternalInput").ap()
    rbW0_d = dt_("rbW0", [H, H], bf16, kind="ExternalInput").ap()
    rbW1_d = dt_("rbW1", [H, H], bf16, kind="ExternalInput").ap()
    Wskip_d = dt_("Wskip", [H, MIN], bf16, kind="ExternalInput").ap()
    raW_d = [dt_(f"raW{i}", [MIN, MIN], bf16, kind="ExternalInput").ap()
             for i in range(4)]
    bias_d = dt_("biases", [P, 8], f32, kind="ExternalInput").ap()
    # col 0: b_tgt, 1: rb_b0, 2: rb_b1, 3: b_skip, 4..7: ra biases

    outT_d = dt_("outT", [MIN, ECP], bf16, kind="ExternalOutput").ap()

    with tile.TileContext(nc) as tc, ExitStack() as ctx:
        const = ctx.enter_context(tc.tile_pool(name="const", bufs=1))

        def load_bf(name, dram_ap, shape):
            t = const.tile(shape, bf16, name=name)
            nc.sync.dma_start(t[:], dram_ap[:])
            return t

        Wdist_sb = load_bf("Wdist", Wdist_d, [NR, H])
        Wsrc_sb = load_bf("Wsrc", Wsrc_d, [MIN, H])
        WbilT_sb = load_bf("WbilT", WbilT_d, [H, BD * H])
        bsrc_sb = load_bf("bsrc", bsrc_d, [1, H])
        Wtgt_sb = load_bf("Wtgt", Wtgt_d, [MIN, H])
        rbW0_sb = load_bf("rbW0", rbW0_d, [H, H])
        rbW1_sb = load_bf("rbW1", rbW1_d, [H, H])
        Wskip_sb = load_bf("Wskip", Wskip_d, [H, MIN])
        raW_sb = [load_bf(f"raW{i}", raW_d[i], [MIN, MIN])
                  for i in range(4)]
        bias_sb = const.tile([P, 8], f32)
        nc.sync.dma_start(bias_sb[:], bias_d[:])

        for _rep in range(repeat):
            with ExitStack() as actx:
                mgs_pool = actx.enter_context(tc.tile_pool(name="mgs", bufs=3))
                sap_pool = actx.enter_context(tc.tile_pool(name="sap", bufs=3))
                sox_pool = actx.enter_context(tc.tile_pool(name="sox", bufs=3))
                dst_pool = actx.enter_context(tc.tile_pool(name="dst", bufs=2))
                sa_pool = actx.enter_context(tc.tile_pool(name="sa", bufs=3))
                gsb_pool = actx.enter_context(tc.tile_pool(name="gsb", bufs=2))
                ghd_pool = actx.enter_context(tc.tile_pool(name="ghd", bufs=2))
                dtb_pool = actx.enter_context(tc.tile_pool(name="dtb", bufs=2))
                ps_big = actx.enter_context(
                    tc.tile_pool(name="ps_big", bufs=2, space="PSUM"))
                ps_sm = actx.enter_context(
                    tc.tile_pool(name="ps_sm", bufs=2, space="PSUM"))
                x0_pool = actx.enter_context(tc.tile_pool(name="x0", bufs=3))
                xb_pool = actx.enter_context(tc.tile_pool(name="xb", bufs=3))
                ps_b = actx.enter_context(
                    tc.tile_pool(name="ps_b", bufs=2, space="PSUM"))

                def silu(ps_in, bias_col):
                    h = xb_pool.tile([P, TB], bf16, name="hsilu", tag="hsilu")
                    nc.scalar.activation(h[:], ps_in[:],
                                         mybir.ActivationFunctionType.Silu,
                                         bias=bias_col, scale=1.0)
                    return h

                # software-pipelined tail: 4 stages, one per group of
                # the following oct, so cross-engine handoffs overlap with
                # a full group of independent work.
                def tail_stage(ts, stage):
                    if stage == 0:
                        csl = slice(ts["c0"], ts["c0"] + TB)
                        x0 = x0_pool.tile([P, TB], bf16, name="x0", tag="x0")
                        nc.sync.dma_start(x0[:], msglocT_d[:, csl])
                        ts["x0"] = x0
                        # p1 = x0 @ Wtgt + agg (agg matmuls fused in)
                        p1 = ps_b.tile([P, TB], f32, space="PSUM", name="p1",
                                       tag="psb")
                        nc.tensor.matmul(p1[:], Wtgt_sb[:], x0[:],
                                         start=True, stop=False,
                                         skip_group_check=True)
                        for bb in range(BD):
                            nc.tensor.matmul(
                                p1[:], WbilT_sb[:, bb * H:(bb + 1) * H],
                                ts["ghdo"][:, bb, :], start=False,
                                stop=(bb == BD - 1), skip_group_check=True)
                        x1 = xb_pool.tile([P, TB], bf16, name="x1", tag="x1")
                        if has_bsrc:
                            nc.scalar.activation(
                                x1[:], p1[:],
                                mybir.ActivationFunctionType.Identity,
                                bias=bias_sb[:, 0:1], scale=1.0)
                        else:
                            nc.scalar.copy(x1[:], p1[:])
                        ts["x1"] = x1
                    elif stage == 1:
                        p2 = ps_b.tile([P, TB], f32, space="PSUM", name="p2",
                                       tag="psb")
                        nc.tensor.matmul(p2[:], rbW0_sb[:], ts["x1"][:],
                                         start=True, stop=True,
                                         skip_group_check=True)
                        ts["h1"] = silu(p2, bias_sb[:, 1:2])
                    elif stage == 2:
                        p3 = ps_b.tile([P, TB], f32, space="PSUM", name="p3",
                                       tag="psb")
                        nc.tensor.matmul(p3[:], rbW1_sb[:], ts["h1"][:],
                                         start=True, stop=True,
                                         skip_group_check=True)
                        ts["h2"] = silu(p3, bias_sb[:, 2:3])
                    elif stage == 3:
                        p4 = ps_b.tile([P, TB], f32, space="PSUM", name="p4",
                                       tag="psb")
                        nc.tensor.matmul(p4[:], Wskip_sb[:], ts["x1"][:],
                                         start=True, stop=False,
                                         skip_group_check=True)
                        nc.tensor.matmul(p4[:], Wskip_sb[:], ts["h2"][:],
                                         start=False, stop=True,
                                         skip_group_check=True)
                        st = silu(p4, bias_sb[:, 3:4])
                        x3 = xb_pool.tile([P, TB], bf16, name="x3", tag="x3")
                        nc.gpsimd.tensor_tensor(out=x3[:], in0=st[:],
                                                in1=ts["x0"][:],
                                                op=mybir.AluOpType.add)
                        ts["xcur"] = x3
                    elif stage in (4, 6):
                        rr = (stage - 4) // 2
                        pa = ps_b.tile([P, TB], f32, space="PSUM",
                                       name=f"pa{rr}", tag="psb")
                        nc.tensor.matmul(pa[:], raW_sb[2 * rr][:],
                                         ts["xcur"][:], start=True, stop=True,
                                         skip_group_check=True)
                        ts["h3"] = silu(pa, bias_sb[:, 4 + 2 * rr:5 + 2 * rr])
                    else:
                        rr = (stage - 5) // 2
                        pb = ps_b.tile([P, TB], f32, space="PSUM",
                                       name=f"pb{rr}", tag="psb")
                        nc.tensor.matmul(pb[:], raW_sb[2 * rr + 1][:],
                                         ts["h3"][:], start=True, stop=True,
                                         skip_group_check=True)
                        h4 = silu(pb, bias_sb[:, 5 + 2 * rr:6 + 2 * rr])
                        xn = xb_pool.tile([P, TB], bf16, name=f"x{4 + rr}",
                                          tag=f"x{4 + rr}")
                        nc.gpsimd.tensor_tensor(out=xn[:], in0=ts["xcur"][:],
                                                in1=h4[:],
                                                op=mybir.AluOpType.add)
                        ts["xcur"] = xn
                        if stage == 7:
                            csl = slice(ts["c0"], ts["c0"] + TB)
                            nc.sync.dma_start(outT_d[:, csl], xn[:])

                def sched_emit(g):
                    for s in range(8):
                        k4 = g - 4 - s
                        if k4 >= 0 and k4 % 4 == 0 and k4 // 4 < NO:
                            k = k4 // 4
                            tail_stage(pend[k], s)
                            if s == 7:
                                del pend[k]

                dstq = None
                dTo = None
                ghdo = None
                pend = {}
                for g in range(NG):
                    og = g % (OB // GB)   # group index within oct (0..3)
                    if og == 0:
                        c0 = g * GB * EB
                        dstq = dst_pool.tile([NR, TB], bf16, name="dstq")
                        nc.sync.dma_start(dstq[:], distT_d[:, c0:c0 + TB])
                        d_ps = ps_sm.tile([P, TB], f32, space="PSUM",
                                          name="d_ps", tag="sm")
                        nc.tensor.matmul(d_ps[:], Wdist_sb[:], dstq[:],
                                         start=True, stop=True,
                                         skip_group_check=True)
                        dTo = dtb_pool.tile([P, TB], bf16, name="dTo")
                        nc.scalar.copy(dTo[:], d_ps[:])
                        ghdo = ghd_pool.tile([P, BD, TB], bf16, name="ghdo")

                    # ---- stream group inputs
                    r0 = g * SLOTG
                    mgs = mgs_pool.tile([P, GL, MIN], bf16, name="mgs")
                    nc.sync.dma_start(
                        mgs[:], msgg_d[r0:r0 + SLOTG, :].rearrange(
                            "(x p) m -> p x m", p=P))
                    sap = sap_pool.tile([P, GL, EB + BD], bf16, name="sap")
                    nc.sync.dma_start(
                        sap[:], sap_d[r0:r0 + SLOTG, :].rearrange(
                            "(x p) m -> p x m", p=P))
                    soh = sap[:, :, 0:EB]     # [P, GL, EB]
                    a_g = sap[:, :, EB:]      # [P, GL, BD]
                    sohx = sox_pool.tile([P, GL, EB * BD], bf16, name="sohx")
                    nc.sync.dma_start(
                        sohx[:], sohx_d[r0:r0 + SLOTG, :].rearrange(
                            "(x p) m -> p x m", p=P))

                    # ---- Sa[j,t,b] = S[j,t] * a[j,b]: contiguous expanded
                    # one-hot => in0/out collapse to linear APs with packed
                    # 2-byte last dims (DVE 2x mode eligible)
                    Sa = sa_pool.tile([P, GL, EB, BD], bf16, name="Sa")
                    eng = (nc.gpsimd if (pool_mod and g % pool_mod
                                         == pool_mod - 1) else nc.vector)
                    eng.tensor_tensor(
                        out=Sa[:],
                        in0=sohx[:, :, :].rearrange("p x (t b) -> p x t b",
                                                    b=BD),
                        in1=a_g[:, :, None, :].to_broadcast([P, GL, EB, BD]),
                        op=mybir.AluOpType.mult)

                    # ---- G: one matmul per (block, sub)
                    G_ps = ps_big.tile([P, GB, EB, BD], f32, space="PSUM",
                                       name="G_ps", tag="big")
                    for bk in range(GB):
                        for s in range(NSUB):
                            nc.tensor.matmul(
                                G_ps[:, bk], mgs[:, bk * NSUB + s, :],
                                Sa[:, bk * NSUB + s], start=(s == 0),
                                stop=(s == NSUB - 1), skip_group_check=True)

                    if has_bsrc:
                        R_ps = ps_sm.tile([BD, GB, EB], f32, space="PSUM",
                                          name="R_ps", tag="sm")
                        for bk in range(GB):
                            for s in range(NSUB):
                                nc.tensor.matmul(
                                    R_ps[:, bk], a_g[:, bk * NSUB + s, :],
                                    soh[:, bk * NSUB + s, :], start=(s == 0),
                                    stop=(s == NSUB - 1),
                                    skip_group_check=True)
                        R_sb = gsb_pool.tile([BD, GB, EB], bf16, name="R_sb")
                        nc.vector.tensor_copy(R_sb[:], R_ps[:])

                    G_sb = gsb_pool.tile([P, GB, EB, BD], bf16, name="G_sb")
                    nc.scalar.copy(G_sb[:], G_ps[:])

                    Gh_ps = ps_big.tile([P, GB, EB, BD], f32, space="PSUM",
                                        name="Gh_ps", tag="big")
                    for hb in range(2):
                        nc.tensor.matmul(
                            Gh_ps[:, hb * 4:(hb + 1) * 4],
                            Wsrc_sb[:],
                            G_sb[:, hb * 4:(hb + 1) * 4],
                            start=True, stop=not has_bsrc,
                            skip_group_check=True)
                    if has_bsrc:
                        for bk in range(GB):
                            for bb in range(BD):
                                nc.tensor.matmul(
                                    Gh_ps[:, bk, :, bb], bsrc_sb[:],
                                    R_sb[bb:bb + 1, bk, :], start=False,
                                    stop=True, skip_group_check=True)

                    # ---- Ghd[h, bk, b, t] = Gh * dT, into the oct tile
                    # ghdo layout [h, b, 512] with col = og*128 + bk*16 + t
                    nc.vector.tensor_tensor(
                        out=ghdo[:, :, og * (GB * EB):(og + 1) * (GB * EB)]
                        .rearrange("p b (k t) -> p k t b", k=GB),
                        in0=Gh_ps[:],
                        in1=dTo[:, og * (GB * EB):(og + 1) * (GB * EB)]
                        .rearrange("p (k t) -> p k t", k=GB)[:, :, :, None]
                        .to_broadcast([P, GB, EB, BD]),
                        op=mybir.AluOpType.mult)

                    if og == (OB // GB) - 1:
                        pend[g // (OB // GB)] = {
                            "c0": (g + 1) * GB * EB - TB, "ghdo": ghdo}
                    sched_emit(g)

                for gv in range(NG, NG + 12):
                    sched_emit(gv)

    nc.compile()
    return nc


# ---------------------------------------------------------------- host prep

def prepare(inputs):
    ai = np.asarray(inputs["angle_index"])
    src = ai[0].astype(np.int64)
    tgt = ai[1].astype(np.int64)
    core = tgt // EC
    loc = tgt - core * EC
    blk = loc // EB
    rel = (loc - blk * EB).astype(np.int64)
    gblk = (core * NB + blk).astype(np.int64)

    order = np.argsort(gblk, kind="stable")
    counts = np.bincount(gblk, minlength=NCORES * NB)
    Lmax = int(counts.max())
    NSUB = max(1, math.ceil(Lmax / P))
    L = NSUB * P

    starts = np.zeros(NCORES * NB + 1, np.int64)
    starts[1:] = np.cumsum(counts)
    gs = gblk[order]
    pos = np.arange(A, dtype=np.int64) - starts[gs]
    dest = gs * L + pos

    SLOT = NCORES * NB * L
    message = np.asarray(inputs["message"])
    distr = np.asarray(inputs["distance_representation"])

    # pre-gathered source messages per slot (pure routing)
    msg_bf = message.astype(bf)
    msgg = np.zeros((SLOT, MIN), bf)
    msgg[dest] = msg_bf[src[order]]

    # packed [S one-hot (EB) | a (BD)] per slot
    sap = np.zeros((SLOT, EB + BD), bf)
    sap[dest, rel[order]] = bf(1.0)
    # expanded one-hot, t-major over b (contiguous for linear DVE walk)
    sohx = np.zeros((SLOT, EB * BD), bf)
    sohx[dest[:, None],
         (rel[order] * BD)[:, None] + np.arange(BD)[None, :]] = bf(1.0)
    angle_flat = np.asarray(inputs["angle_representation"]).reshape(A, NS * NR)
    a_host = (angle_flat @ np.asarray(inputs["W_angle"])).astype(bf)
    sap[dest, EB:] = a_host[order]

    Wdist = np.asarray(inputs["W_dist"]).astype(bf)
    Wsrc = np.asarray(inputs["W_src"]).astype(bf)
    WbilT = np.ascontiguousarray(
        np.asarray(inputs["W_bil"]).transpose(2, 1, 0).reshape(H, BD * H)
    ).astype(bf)
    bsrc = np.asarray(inputs["b_src"]).astype(np.float32)
    has_bsrc = bool(np.any(bsrc != 0) or np.any(np.asarray(inputs["b_tgt"]) != 0)
                    or np.any(np.asarray(inputs["res_before_b"]) != 0)
                    or np.any(np.asarray(inputs["b_skip"]) != 0)
                    or np.any(np.asarray(inputs["res_after_b"]) != 0))

    biases = np.zeros((P, 8), np.float32)
    biases[:, 0] = np.asarray(inputs["b_tgt"])
    biases[:, 1] = np.asarray(inputs["res_before_b"])[0, 0]
    biases[:, 2] = np.asarray(inputs["res_before_b"])[0, 1]
    biases[:, 3] = np.asarray(inputs["b_skip"])
    biases[:, 4] = np.asarray(inputs["res_after_b"])[0, 0]
    biases[:, 5] = np.asarray(inputs["res_after_b"])[0, 1]
    biases[:, 6] = np.asarray(inputs["res_after_b"])[1, 0]
    biases[:, 7] = np.asarray(inputs["res_after_b"])[1, 1]

    shared = dict(
        Wdist=Wdist, Wsrc=Wsrc, WbilT=WbilT,
        bsrc=np.ascontiguousarray(bsrc[None, :]).astype(bf),
        Wtgt=np.asarray(inputs["W_tgt"]).astype(bf),
        rbW0=np.asarray(inputs["res_before_W"])[0, 0].astype(bf),
        rbW1=np.asarray(inputs["res_before_W"])[0, 1].astype(bf),
        Wskip=np.asarray(inputs["W_skip"]).astype(bf),
        raW0=np.asarray(inputs["res_after_W"])[0, 0].astype(bf),
        raW1=np.asarray(inputs["res_after_W"])[0, 1].astype(bf),
        raW2=np.asarray(inputs["res_after_W"])[1, 0].astype(bf),
        raW3=np.asarray(inputs["res_after_W"])[1, 1].astype(bf),
        biases=biases,
    )

    in_maps = []
    SLOTC = NB * L
    for c in range(NCORES):
        s0 = c * SLOTC
        dr = np.zeros((ECP, NR), bf)
        dr[:EC] = distr[c * EC:(c + 1) * EC].astype(bf)
        distT = np.ascontiguousarray(dr.T)
        ml = np.zeros((ECP, MIN), bf)
        ml[:EC] = message[c * EC:(c + 1) * EC].astype(bf)
        msglocT = np.ascontiguousarray(ml.T)
        in_maps.append(dict(shared,
                            msgg=msgg[s0:s0 + SLOTC],
                            sap=sap[s0:s0 + SLOTC],
                            sohx=sohx[s0:s0 + SLOTC],
                            distT=distT, msglocT=msglocT))
    return in_maps, NSUB, has_bsrc


# ---------------------------------------------------------------- runner

def make_runner(nc, n_cores):
    """jit-compiled PJRT runner for a prebuilt nc; returns fn(in_maps)->outs."""
    import jax
    from jax.sharding import Mesh, PartitionSpec, NamedSharding
    from jax.experimental.shard_map import shard_map
    from concourse.bass2jax import (_bass_exec_p, install_neuronx_cc_hook,
                                    partition_id_tensor)

    install_neuronx_cc_hook()
    partition_name = (nc.partition_id_tensor.name
                      if nc.partition_id_tensor else None)
    in_names, out_names, out_avals, zero_shapes = [], [], [], []
    for alloc in nc.m.functions[0].allocations:
        if not isinstance(alloc, mybir.MemoryLocationSet):
            continue
        name = alloc.memorylocations[0].name
        if alloc.kind == "ExternalInput":
            if name != partition_name:
                in_names.append(name)
        elif alloc.kind == "ExternalOutput":
            out_names.append(name)
            shape = tuple(alloc.tensor_shape)
            dtype = mybir.dt.np(alloc.dtype)
            out_avals.append(jax.core.ShapedArray(shape, dtype))
            zero_shapes.append((shape, dtype))
    n_params = len(in_names)
    n_outs = len(out_avals)
    all_in_names = in_names + out_names + (
        [partition_name] if partition_name else [])

    def _body(*args):
        operands = list(args)
        if partition_name is not None:
            operands.append(partition_id_tensor())
        outs = _bass_exec_p.bind(
            *operands, out_avals=tuple(out_avals), in_names=tuple(all_in_names),
            out_names=tuple(out_names), lowering_input_output_aliases=(),
            sim_require_finite=False, sim_require_nnan=False, nc=nc)
        return tuple(outs)

    donate = tuple(range(n_params, n_params + n_outs))
    devices = jax.devices()[:n_cores]
    mesh = Mesh(np.asarray(devices), ("core",))
    sharded = jax.jit(
        shard_map(_body, mesh=mesh,
                  in_specs=(PartitionSpec("core"),) * (n_params + n_outs),
                  out_specs=(PartitionSpec("core"),) * n_outs,
                  check_rep=False),
        donate_argnums=donate, keep_unused=True)
    shard = NamedSharding(mesh, PartitionSpec("core"))

    def put_inputs(in_maps):
        import jax
        return [jax.device_put(
            np.concatenate([np.asarray(m[n]) for m in in_maps], axis=0), shard)
            for n in in_names]

    def zeros():
        import jax
        return [jax.device_put(
            np.zeros((n_cores * s[0], *s[1:]), d), shard)
            for (s, d) in zero_shapes]

    def run(dev_ins, zbufs=None):
        import jax
        outs = sharded(*dev_ins, *(zbufs if zbufs is not None else zeros()))
        jax.block_until_ready(outs)
        return {n: np.asarray(outs[i]).reshape(n_cores, *out_avals[i].shape)
                for i, n in enumerate(out_names)}

    run.zeros = zeros
    return run, put_inputs


_cache = {}


def _get_built(NSUB, has_bsrc, repeat=1):
    key = (NSUB, has_bsrc, repeat)
    if key not in _cache:
        nc = build_nc(NSUB, has_bsrc, repeat=repeat)
        run, put = make_runner(nc, NCORES)
        _cache[key] = (run, put)
    return _cache[key]


def kernel(**inputs) -> np.ndarray:
    in_maps, NSUB, has_bsrc = prepare(inputs)
    run, put = _get_built(NSUB, has_bsrc)
    dev_ins = put(in_maps)
    outs = run(dev_ins)
    outT = outs["outT"]  # [NCORES, MIN, ECP]
    out = np.concatenate([outT[c].T[:EC] for c in range(NCORES)], axis=0)
    return out.astype(np.float32)


# revision 26
# speedup vs baseline: 2.7803x; 2.7803x over previous
"""DimeNet edge-update kernel for 8 Trainium2 NeuronCores (v3).

Strategy (graph/data parallel, per the sharding hint):
  - Edges are split into 8 contiguous ranges of 25000 (one per core).
  - Angle triplets are routed (on host) to the core owning their TARGET edge,
    sorted by target, grouped into blocks of EB=16 consecutive target edges.
    With EB=16 a block holds ~80 angles on average (max ~115), so a single
    128-slot sub-block covers a block with no multi-sub accumulation.
  - Host routing also pre-gathers source messages per slot (msgg), builds the
    one-hot scatter S (slot -> target-within-block), and evaluates the tiny
    42->8 angle projection a = ang @ W_angle (0.25% of model FLOPs); S and a
    are packed together (sa_pack).  All heavy FLOPs stay on device.
  - Blocks are processed in GROUPS of 8 (one wide op each for Sa / Gh / Ghd)
    and OCTS of 32 (= 512 edges, one tail tile):
        Sa[j,bk,b,t] = a[j,bk,b] * S[j,bk,t]          (DVE/Pool, 1 op/group)
        G[k,bk,(b,t)] = sum_j msgg[j,k] Sa[j,bk,b,t]  (PE, 1 mm/block)
        Gh[h,...]    = Wsrc^T-contraction of G        (PE, 2 mm/group)
        Ghd          = Gh * dT (d = dist @ Wdist)     (DVE, 1 op/group)
        p1           = Wtgt@x0 + sum_b WbilT_b@Ghd_b  (PE, fused into tail)
    which equals agg + message @ W_tgt with
    agg = segment_sum(einsum('ab,ah,ibh->ai', a, sm, W_bil), tgt),
    sm = (msg[src] @ W_src + b_src) * d[tgt].
  - The edge-wise tail MLP runs fused in bf16 at N=512 tiles,
    software-pipelined in 8 stages across the two following octs so the
    serial PE->Act->Pool dependency chain overlaps with group work.
"""

import sys

sys.path.insert(0, "/opt/trn_rl_repo")

import math
from contextlib import ExitStack

import numpy as np
import ml_dtypes

import concourse.bass as bass
import concourse.tile as tile
from concourse import bacc, mybir

f32 = mybir.dt.float32
f32r = mybir.dt.float32r
bf16 = mybir.dt.bfloat16
i32 = mybir.dt.int32
bf = ml_dtypes.bfloat16

E = 200000
A = 1000000
H = 128
BD = 8
NR = 6
NS = 7
MIN = 128
NCORES = 8
EC = E // NCORES          # 25000 edges per core
EB = 16                   # edges per block
GB = 8                    # blocks per group
OB = 32                   # blocks per oct (= tail tile of 512 edges)
NB = 1568                 # blocks per core (25088 edges padded)
ECP = NB * EB             # 25088
NG = NB // GB             # 196 groups
NO = NB // OB             # 49 octs
P = 128
TB = 512


# ---------------------------------------------------------------- device build

def build_nc(NSUB, has_bsrc, repeat=1, num_devices=NCORES, pool_mod=0):
    """pool_mod: every pool_mod-th group's Sa product runs on gpsimd (Pool);
    0 disables Pool offload."""
    GL = GB * NSUB            # sub-slots per group
    SLOTG = GL * P            # angle slots per group
    nc = bacc.Bacc("TRN2", target_bir_lowering=False, debug=False,
                   enable_asserts=False, num_devices=num_devices)

    dt_ = nc.dram_tensor
    msgg_d = dt_("msgg", [NG * SLOTG, MIN], bf16, kind="ExternalInput").ap()
    sap_d = dt_("sap", [NG * SLOTG, EB + BD], bf16, kind="ExternalInput").ap()
    distT_d = dt_("distT", [NR, ECP], bf16, kind="ExternalInput").ap()
    msglocT_d = dt_("msglocT", [MIN, ECP], bf16, kind="ExternalInput").ap()
    Wdist_d = dt_("Wdist", [NR, H], bf16, kind="ExternalInput").ap()
    Wsrc_d = dt_("Wsrc", [MIN, H], bf16, kind="ExternalInput").ap()
    WbilT_d = dt_("WbilT", [H, BD * H], bf16, kind="ExternalInput").ap()
    bsrc_d = dt_("bsrc", [1, H], bf16, kind="ExternalInput").ap()
    Wtgt_d = dt_("Wtgt", [MIN, H], bf16, kind="ExternalInput").ap()
    rbW0_d = dt_("rbW0", [H, H], bf16, kind="ExternalInput").ap()
    rbW1_d = dt_("rbW1", [H, H], bf16, kind="ExternalInput").ap()
    Wskip_d = dt_("Wskip", [H, MIN], bf16, kind="ExternalInput").ap()
    raW_d = [dt_(f"raW{i}", [MIN, MIN], bf16, kind="ExternalInput").ap()
             for i in range(4)]
    bias_d = dt_("biases", [P, 8], f32, kind="ExternalInput").ap()
    # col 0: b_tgt, 1: rb_b0, 2: rb_b1, 3: b_skip, 4..7: ra biases

    outT_d = dt_("outT", [MIN, ECP], bf16, kind="ExternalOutput").ap()

    with tile.TileContext(nc) as tc, ExitStack() as ctx:
        const = ctx.enter_context(tc.tile_pool(name="const", bufs=1))

        def load_bf(name, dram_ap, shape):
            t = const.tile(shape, bf16, name=name)
            nc.sync.dma_start(t[:], dram_ap[:])
            return t

        Wdist_sb = load_bf("Wdist", Wdist_d, [NR, H])
        Wsrc_sb = load_bf("Wsrc", Wsrc_d, [MIN, H])
        WbilT_sb = load_bf("WbilT", WbilT_d, [H, BD * H])
        bsrc_sb = load_bf("bsrc", bsrc_d, [1, H])
        Wtgt_sb = load_bf("Wtgt", Wtgt_d, [MIN, H])
        rbW0_sb = load_bf("rbW0", rbW0_d, [H, H])
        rbW1_sb = load_bf("rbW1", rbW1_d, [H, H])
        Wskip_sb = load_bf("Wskip", Wskip_d, [H, MIN])
        raW_sb = [load_bf(f"raW{i}", raW_d[i], [MIN, MIN])
                  for i in range(4)]
        bias_sb = const.tile([P, 8], f32)
        nc.sync.dma_start(bias_sb[:], bias_d[:])

        for _rep in range(repeat):
            with ExitStack() as actx:
                mgs_pool = actx.enter_context(tc.tile_pool(name="mgs", bufs=3))
                sap_pool = actx.enter_context(tc.tile_pool(name="sap", bufs=3))
                dst_pool = actx.enter_context(tc.tile_pool(name="dst", bufs=2))
                sa_pool = actx.enter_context(tc.tile_pool(name="sa", bufs=3))
                gsb_pool = actx.enter_context(tc.tile_pool(name="gsb", bufs=2))
                ghd_pool = actx.enter_context(tc.tile_pool(name="ghd", bufs=2))
                dtb_pool = actx.enter_context(tc.tile_pool(name="dtb", bufs=2))
                ps_big = actx.enter_context(
                    tc.tile_pool(name="ps_big", bufs=2, space="PSUM"))
                ps_sm = actx.enter_context(
                    tc.tile_pool(name="ps_sm", bufs=2, space="PSUM"))
                x0_pool = actx.enter_context(tc.tile_pool(name="x0", bufs=3))
                xb_pool = actx.enter_context(tc.tile_pool(name="xb", bufs=3))
                ps_b = actx.enter_context(
                    tc.tile_pool(name="ps_b", bufs=2, space="PSUM"))

                def silu(ps_in, bias_col):
                    h = xb_pool.tile([P, TB], bf16, name="hsilu", tag="hsilu")
                    nc.scalar.activation(h[:], ps_in[:],
                                         mybir.ActivationFunctionType.Silu,
                                         bias=bias_col, scale=1.0)
                    return h

                # software-pipelined tail: 4 stages, one per group of
                # the following oct, so cross-engine handoffs overlap with
                # a full group of independent work.
                def tail_stage(ts, stage):
                    if stage == 0:
                        csl = slice(ts["c0"], ts["c0"] + TB)
                        x0 = x0_pool.tile([P, TB], bf16, name="x0", tag="x0")
                        nc.sync.dma_start(x0[:], msglocT_d[:, csl])
                        ts["x0"] = x0
                        # p1 = x0 @ Wtgt + agg (agg matmuls fused in)
                        p1 = ps_b.tile([P, TB], f32, space="PSUM", name="p1",
                                       tag="psb")
                        nc.tensor.matmul(p1[:], Wtgt_sb[:], x0[:],
                                         start=True, stop=False,
                                         skip_group_check=True)
                        for bb in range(BD):
                            nc.tensor.matmul(
                                p1[:], WbilT_sb[:, bb * H:(bb + 1) * H],
                                ts["ghdo"][:, bb, :], start=False,
                                stop=(bb == BD - 1), skip_group_check=True)
                        x1 = xb_pool.tile([P, TB], bf16, name="x1", tag="x1")
                        if has_bsrc:
                            nc.scalar.activation(
                                x1[:], p1[:],
                                mybir.ActivationFunctionType.Identity,
                                bias=bias_sb[:, 0:1], scale=1.0)
                        else:
                            nc.scalar.copy(x1[:], p1[:])
                        ts["x1"] = x1
                    elif stage == 1:
                        p2 = ps_b.tile([P, TB], f32, space="PSUM", name="p2",
                                       tag="psb")
                        nc.tensor.matmul(p2[:], rbW0_sb[:], ts["x1"][:],
                                         start=True, stop=True,
                                         skip_group_check=True)
                        ts["h1"] = silu(p2, bias_sb[:, 1:2])
                    elif stage == 2:
                        p3 = ps_b.tile([P, TB], f32, space="PSUM", name="p3",
                                       tag="psb")
                        nc.tensor.matmul(p3[:], rbW1_sb[:], ts["h1"][:],
                                         start=True, stop=True,
                                         skip_group_check=True)
                        ts["h2"] = silu(p3, bias_sb[:, 2:3])
                    elif stage == 3:
                        p4 = ps_b.tile([P, TB], f32, space="PSUM", name="p4",
                                       tag="psb")
                        nc.tensor.matmul(p4[:], Wskip_sb[:], ts["x1"][:],
                                         start=True, stop=False,
                                         skip_group_check=True)
                        nc.tensor.matmul(p4[:], Wskip_sb[:], ts["h2"][:],
                                         start=False, stop=True,
                                         skip_group_check=True)
                        st = silu(p4, bias_sb[:, 3:4])
                        x3 = xb_pool.tile([P, TB], bf16, name="x3", tag="x3")
                        nc.gpsimd.tensor_tensor(out=x3[:], in0=st[:],
                                                in1=ts["x0"][:],
                                                op=mybir.AluOpType.add)
                        ts["xcur"] = x3
                    elif stage in (4, 6):
                        rr = (stage - 4) // 2
                        pa = ps_b.tile([P, TB], f32, space="PSUM",
                                       name=f"pa{rr}", tag="psb")
                        nc.tensor.matmul(pa[:], raW_sb[2 * rr][:],
                                         ts["xcur"][:], start=True, stop=True,
                                         skip_group_check=True)
                        ts["h3"] = silu(pa, bias_sb[:, 4 + 2 * rr:5 + 2 * rr])
                    else:
                        rr = (stage - 5) // 2
                        pb = ps_b.tile([P, TB], f32, space="PSUM",
                                       name=f"pb{rr}", tag="psb")
                        nc.tensor.matmul(pb[:], raW_sb[2 * rr + 1][:],
                                         ts["h3"][:], start=True, stop=True,
                                         skip_group_check=True)
                        h4 = silu(pb, bias_sb[:, 5 + 2 * rr:6 + 2 * rr])
                        xn = xb_pool.tile([P, TB], bf16, name=f"x{4 + rr}",
                                          tag=f"x{4 + rr}")
                        nc.gpsimd.tensor_tensor(out=xn[:], in0=ts["xcur"][:],
                                                in1=h4[:],
                                                op=mybir.AluOpType.add)
                        ts["xcur"] = xn
                        if stage == 7:
                            csl = slice(ts["c0"], ts["c0"] + TB)
                            nc.sync.dma_start(outT_d[:, csl], xn[:])

                def sched_emit(g):
                    for s in range(8):
                        k4 = g - 4 - s
                        if k4 >= 0 and k4 % 4 == 0 and k4 // 4 < NO:
                            k = k4 // 4
                            tail_stage(pend[k], s)
                            if s == 7:
                                del pend[k]

                dstq = None
                dTo = None
                ghdo = None
                pend = {}
                for g in range(NG):
                    og = g % (OB // GB)   # group index within oct (0..3)
                    if og == 0:
                        c0 = g * GB * EB
                        dstq = dst_pool.tile([NR, TB], bf16, name="dstq")
                        nc.sync.dma_start(dstq[:], distT_d[:, c0:c0 + TB])
                        d_ps = ps_sm.tile([P, TB], f32, space="PSUM",
                                          name="d_ps", tag="sm")
                        nc.tensor.matmul(d_ps[:], Wdist_sb[:], dstq[:],
                                         start=True, stop=True,
                                         skip_group_check=True)
                        dTo = dtb_pool.tile([P, TB], bf16, name="dTo")
                        nc.scalar.copy(dTo[:], d_ps[:])
                        ghdo = ghd_pool.tile([P, BD, TB], bf16, name="ghdo")

                    # ---- stream group inputs
                    r0 = g * SLOTG
                    mgs = mgs_pool.tile([P, GL, MIN], bf16, name="mgs")
                    nc.sync.dma_start(
                        mgs[:], msgg_d[r0:r0 + SLOTG, :].rearrange(
                            "(x p) m -> p x m", p=P))
                    sap = sap_pool.tile([P, GL, EB + BD], bf16, name="sap")
                    nc.sync.dma_start(
                        sap[:], sap_d[r0:r0 + SLOTG, :].rearrange(
                            "(x p) m -> p x m", p=P))
                    soh = sap[:, :, 0:EB]     # [P, GL, EB]
                    a_g = sap[:, :, EB:]      # [P, GL, BD]

                    # ---- Sa = S (x) a : one wide broadcast multiply
                    Sa = sa_pool.tile([P, GL, BD, EB], bf16, name="Sa")
                    eng = (nc.gpsimd if (pool_mod and g % pool_mod
                                         == pool_mod - 1) else nc.vector)
                    eng.tensor_tensor(
                        out=Sa[:],
                        in0=soh[:, :, None, :].to_broadcast([P, GL, BD, EB]),
                        in1=a_g[:, :, :, None].to_broadcast([P, GL, BD, EB]),
                        op=mybir.AluOpType.mult)

                    # ---- G: one matmul per (block, sub)
                    G_ps = ps_big.tile([P, GB, BD, EB], f32, space="PSUM",
                                       name="G_ps", tag="big")
                    for bk in range(GB):
                        for s in range(NSUB):
                            nc.tensor.matmul(
                                G_ps[:, bk], mgs[:, bk * NSUB + s, :],
                                Sa[:, bk * NSUB + s], start=(s == 0),
                                stop=(s == NSUB - 1), skip_group_check=True)

                    if has_bsrc:
                        R_ps = ps_sm.tile([BD, GB, EB], f32, space="PSUM",
                                          name="R_ps", tag="sm")
                        for bk in range(GB):
                            for s in range(NSUB):
                                nc.tensor.matmul(
                                    R_ps[:, bk], a_g[:, bk * NSUB + s, :],
                                    soh[:, bk * NSUB + s, :], start=(s == 0),
                                    stop=(s == NSUB - 1),
                                    skip_group_check=True)
                        R_sb = gsb_pool.tile([BD, GB, EB], bf16, name="R_sb")
                        nc.vector.tensor_copy(R_sb[:], R_ps[:])

                    G_sb = gsb_pool.tile([P, GB, BD, EB], bf16, name="G_sb")
                    nc.scalar.copy(G_sb[:], G_ps[:])

                    Gh_ps = ps_big.tile([P, GB, BD, EB], f32, space="PSUM",
                                        name="Gh_ps", tag="big")
                    for hb in range(2):
                        nc.tensor.matmul(
                            Gh_ps[:, hb * 4:(hb + 1) * 4],
                            Wsrc_sb[:],
                            G_sb[:, hb * 4:(hb + 1) * 4],
                            start=True, stop=not has_bsrc,
                            skip_group_check=True)
                    if has_bsrc:
                        for bk in range(GB):
                            for bb in range(BD):
                                nc.tensor.matmul(
                                    Gh_ps[:, bk, bb, :], bsrc_sb[:],
                                    R_sb[bb:bb + 1, bk, :], start=False,
                                    stop=True, skip_group_check=True)

                    # ---- Ghd[h, bk, b, t] = Gh * dT, into the oct tile
                    # ghdo layout [h, b, 512] with col = og*128 + bk*16 + t
                    nc.vector.tensor_tensor(
                        out=ghdo[:, :, og * (GB * EB):(og + 1) * (GB * EB)]
                        .rearrange("p b (k t) -> p k b t", k=GB),
                        in0=Gh_ps[:],
                        in1=dTo[:, og * (GB * EB):(og + 1) * (GB * EB)]
                        .rearrange("p (k t) -> p k t", k=GB)[:, :, None, :]
                        .to_broadcast([P, GB, BD, EB]),
                        op=mybir.AluOpType.mult)

                    if og == (OB // GB) - 1:
                        pend[g // (OB // GB)] = {
                            "c0": (g + 1) * GB * EB - TB, "ghdo": ghdo}
                    sched_emit(g)

                for gv in range(NG, NG + 12):
                    sched_emit(gv)

    nc.compile()
    return nc


# ---------------------------------------------------------------- host prep

def prepare(inputs):
    ai = np.asarray(inputs["angle_index"])
    src = ai[0].astype(np.int64)
    tgt = ai[1].astype(np.int64)
    core = tgt // EC
    loc = tgt - core * EC
    blk = loc // EB
    rel = (loc - blk * EB).astype(np.int64)
    gblk = (core * NB + blk).astype(np.int64)

    order = np.argsort(gblk, kind="stable")
    counts = np.bincount(gblk, minlength=NCORES * NB)
    Lmax = int(counts.max())
    NSUB = max(1, math.ceil(Lmax / P))
    L = NSUB * P

    starts = np.zeros(NCORES * NB + 1, np.int64)
    starts[1:] = np.cumsum(counts)
    gs = gblk[order]
    pos = np.arange(A, dtype=np.int64) - starts[gs]
    dest = gs * L + pos

    SLOT = NCORES * NB * L
    message = np.asarray(inputs["message"])
    distr = np.asarray(inputs["distance_representation"])

    # pre-gathered source messages per slot (pure routing)
    msg_bf = message.astype(bf)
    msgg = np.zeros((SLOT, MIN), bf)
    msgg[dest] = msg_bf[src[order]]

    # packed [S one-hot (EB) | a (BD)] per slot
    sap = np.zeros((SLOT, EB + BD), bf)
    sap[dest, rel[order]] = bf(1.0)
    angle_flat = np.asarray(inputs["angle_representation"]).reshape(A, NS * NR)
    a_host = (angle_flat @ np.asarray(inputs["W_angle"])).astype(bf)
    sap[dest, EB:] = a_host[order]

    Wdist = np.asarray(inputs["W_dist"]).astype(bf)
    Wsrc = np.asarray(inputs["W_src"]).astype(bf)
    WbilT = np.ascontiguousarray(
        np.asarray(inputs["W_bil"]).transpose(2, 1, 0).reshape(H, BD * H)
    ).astype(bf)
    bsrc = np.asarray(inputs["b_src"]).astype(np.float32)
    has_bsrc = bool(np.any(bsrc != 0) or np.any(np.asarray(inputs["b_tgt"]) != 0)
                    or np.any(np.asarray(inputs["res_before_b"]) != 0)
                    or np.any(np.asarray(inputs["b_skip"]) != 0)
                    or np.any(np.asarray(inputs["res_after_b"]) != 0))

    biases = np.zeros((P, 8), np.float32)
    biases[:, 0] = np.asarray(inputs["b_tgt"])
    biases[:, 1] = np.asarray(inputs["res_before_b"])[0, 0]
    biases[:, 2] = np.asarray(inputs["res_before_b"])[0, 1]
    biases[:, 3] = np.asarray(inputs["b_skip"])
    biases[:, 4] = np.asarray(inputs["res_after_b"])[0, 0]
    biases[:, 5] = np.asarray(inputs["res_after_b"])[0, 1]
    biases[:, 6] = np.asarray(inputs["res_after_b"])[1, 0]
    biases[:, 7] = np.asarray(inputs["res_after_b"])[1, 1]

    shared = dict(
        Wdist=Wdist, Wsrc=Wsrc, WbilT=WbilT,
        bsrc=np.ascontiguousarray(bsrc[None, :]).astype(bf),
        Wtgt=np.asarray(inputs["W_tgt"]).astype(bf),
        rbW0=np.asarray(inputs["res_before_W"])[0, 0].astype(bf),
        rbW1=np.asarray(inputs["res_before_W"])[0, 1].astype(bf),
        Wskip=np.asarray(inputs["W_skip"]).astype(bf),
        raW0=np.asarray(inputs["res_after_W"])[0, 0].astype(bf),
        raW1=np.asarray(inputs["res_after_W"])[0, 1].astype(bf),
        raW2=np.asarray(inputs["res_after_W"])[1, 0].astype(bf),
        raW3=np.asarray(inputs["res_after_W"])[1, 1].astype(bf),
        biases=biases,
    )

    in_maps = []
    SLOTC = NB * L
    for c in range(NCORES):
        s0 = c * SLOTC
        dr = np.zeros((ECP, NR), bf)
        dr[:EC] = distr[c * EC:(c + 1) * EC].astype(bf)
        distT = np.ascontiguousarray(dr.T)
        ml = np.zeros((ECP, MIN), bf)
        ml[:EC] = message[c * EC:(c + 1) * EC].astype(bf)
        msglocT = np.ascontiguousarray(ml.T)
        in_maps.append(dict(shared,
                            msgg=msgg[s0:s0 + SLOTC],
                            sap=sap[s0:s0 + SLOTC],
                            distT=distT, msglocT=msglocT))
    return in_maps, NSUB, has_bsrc


# ---------------------------------------------------------------- runner

def make_runner(nc, n_cores):
    """jit-compiled PJRT runner for a prebuilt nc; returns fn(in_maps)->outs."""
    import jax
    from jax.sharding import Mesh, PartitionSpec, NamedSharding
    from jax.experimental.shard_map import shard_map
    from concourse.bass2jax import (_bass_exec_p, install_neuronx_cc_hook,
                                    partition_id_tensor)

    install_neuronx_cc_hook()
    partition_name = (nc.partition_id_tensor.name
                      if nc.partition_id_tensor else None)
    in_names, out_names, out_avals, zero_shapes = [], [], [], []
    for alloc in nc.m.functions[0].allocations:
        if not isinstance(alloc, mybir.MemoryLocationSet):
            continue
        name = alloc.memorylocations[0].name
        if alloc.kind == "ExternalInput":
            if name != partition_name:
                in_names.append(name)
        elif alloc.kind == "ExternalOutput":
            out_names.append(name)
            shape = tuple(alloc.tensor_shape)
            dtype = mybir.dt.np(alloc.dtype)
            out_avals.append(jax.core.ShapedArray(shape, dtype))
            zero_shapes.append((shape, dtype))
    n_params = len(in_names)
    n_outs = len(out_avals)
    all_in_names = in_names + out_names + (
        [partition_name] if partition_name else [])

    def _body(*args):
        operands = list(args)
        if partition_name is not None:
            operands.append(partition_id_tensor())
        outs = _bass_exec_p.bind(
            *operands, out_avals=tuple(out_avals), in_names=tuple(all_in_names),
            out_names=tuple(out_names), lowering_input_output_aliases=(),
            sim_require_finite=False, sim_require_nnan=False, nc=nc)
        return tuple(outs)

    donate = tuple(range(n_params, n_params + n_outs))
    devices = jax.devices()[:n_cores]
    mesh = Mesh(np.asarray(devices), ("core",))
    sharded = jax.jit(
        shard_map(_body, mesh=mesh,
                  in_specs=(PartitionSpec("core"),) * (n_params + n_outs),
                  out_specs=(PartitionSpec("core"),) * n_outs,
                  check_rep=False),
        donate_argnums=donate, keep_unused=True)
    shard = NamedSharding(mesh, PartitionSpec("core"))

    def put_inputs(in_maps):
        import jax
        return [jax.device_put(
            np.concatenate([np.asarray(m[n]) for m in in_maps], axis=0), shard)
            for n in in_names]

    def zeros():
        import jax
        return [jax.device_put(
            np.zeros((n_cores * s[0], *s[1:]), d), shard)
            for (s, d) in zero_shapes]

    def run(dev_ins, zbufs=None):
        import jax
        outs = sharded(*dev_ins, *(zbufs if zbufs is not None else zeros()))
        jax.block_until_ready(outs)
        return {n: np.asarray(outs[i]).reshape(n_cores, *out_avals[i].shape)
                for i, n in enumerate(out_names)}

    run.zeros = zeros
    return run, put_inputs


_cache = {}


def _get_built(NSUB, has_bsrc, repeat=1):
    key = (NSUB, has_bsrc, repeat)
    if key not in _cache:
        nc = build_nc(NSUB, has_bsrc, repeat=repeat)
        run, put = make_runner(nc, NCORES)
        _cache[key] = (run, put)
    return _cache[key]


def kernel(**inputs) -> np.ndarray:
    in_maps, NSUB, has_bsrc = prepare(inputs)
    run, put = _get_built(NSUB, has_bsrc)
    dev_ins = put(in_maps)
    outs = run(dev_ins)
    outT = outs["outT"]  # [NCORES, MIN, ECP]
    out = np.concatenate([outT[c].T[:EC] for c in range(NCORES)], axis=0)
    return out.astype(np.float32)
